# revision 1
# baseline (speedup 1.0000x reference)
"""Causal self-attention head (softmax over the QUERY axis) on 8 trn2 cores.

Reference math (note the unusual softmax axis=-2, i.e. per key-column):
    q = x @ Wq; k = x @ Wk; v = x @ Wv            # [B,T,64]
    s[b,q,k] = (q . k) * 64**-0.5, masked to q >= k
    w[:, k]  = softmax over q of s[:, k]           # column softmax
    out[b,q,:] = sum_k w[q,k] v[k,:]

Because the softmax normalizes over q (the contraction axis of the second
matmul is k), the normalizer folds into a per-key scaling of v:
    out[q] = sum_{k<=q} exp(s[q,k]) * (r[k] * v[k]),  r[k] = 1/sum_{q>=k} exp(s[q,k])

Sharding: 8 cores = 4 batches x 2 "parities". Core (b, p) owns the key-column
blocks kb = 2i+p (i=0..15, 128 columns each) of batch b and produces a partial
output over all q; the host adds the two parity partials per batch.

To keep the SPMD program identical on all cores, parity-1 cores receive x^T
shifted left by 128 columns (zero-padded tail); their key blocks then sit at
the same compile-time offsets as parity-0's, and the host shifts their output
back by +128 q positions. Garbage q-columns from the zero pad are killed by a
per-core "tailmask" input.

Pipeline: x^T chunks are processed in DESCENDING order, so each chunk's two
key blocks can run their scores+exp immediately (they only need q^T columns
from chunks already loaded). The ACT engine's exp work therefore overlaps the
projection matmuls from the start, keeping the PE dense (HAM clock-gate warm).
The output chunks run as a dense PE tail.

Other notes:
- The chunk's two key blocks are projected with ONE matmul per contraction
  subtile using a strided 3D moving-operand access pattern (N=256); k and v
  share one psum bank.
- Causal masks are applied by accumulating triangular-count matmuls into the
  scores PSUM (PE-local).
- exp runs on ACT over [128,1024] psum tiles with the 1/sqrt(64) folded into
  its free affine pre-scale and per-column sums from fused accum_out.
- v^T -> v-natural transposes run on the DMA xbar (bf16), costing no PE/PSUM.
- PSUM->SBUF copies run on the vector engine (ACT is the exp bottleneck).
"""

import os
import sys

import numpy as np

for _p in ("/opt/trn_rl_repo",):
    if _p not in sys.path:
        sys.path.insert(0, _p)

import concourse.bass as bass
import concourse.mybir as mybir
from concourse import bacc
from concourse.bass_utils import run_bass_kernel_spmd
from concourse.tile import TileContext

B, T, CE, CH = 4, 4096, 1024, 64
P = 128
NB = 16          # key blocks per core (128 cols each)
NCHUNK = 8       # 512-col chunks covering T
SCALE = CH ** -0.5
NEG = -1e30
M0 = NEG / P     # per-unit mask magnitude for the triangular-count mask
ETILE = 1024     # scores/exp tile width (2 psum banks)

F32 = mybir.dt.float32
BF16 = mybir.dt.bfloat16

N_CORES = 8

# Results of the last run (for test harnesses: exec_time_ns etc.)
LAST_RESULTS = None


def _build_program():
    # Bacc (not plain Bass): its compile() pipeline legalizes multi-semaphore
    # waits into EventSemaphore instructions and moves matmul waits onto
    # LDWEIGHTS — required by the 1-wait-per-instruction hardware encoding.
    nc = bacc.Bacc("TRN2", target_bir_lowering=False, debug=False)

    xT = nc.declare_dram_parameter("xT", [CE, T], BF16, isOutput=False)
    wq = nc.declare_dram_parameter("wq", [CE, CH], BF16, isOutput=False)
    wk = nc.declare_dram_parameter("wk", [CE, CH], BF16, isOutput=False)
    wv = nc.declare_dram_parameter("wv", [CE, CH], BF16, isOutput=False)
    tailmask = nc.declare_dram_parameter("tailmask", [P, P], BF16, isOutput=False)
    outT = nc.declare_dram_parameter("outT", [CH, T], F32, isOutput=True)

    with TileContext(nc) as tc:
        with (
            tc.tile_pool(name="consts", bufs=1) as consts,
            tc.tile_pool(name="qkv", bufs=1) as qkv,
            tc.tile_pool(name="w2p", bufs=1) as w2p,
            tc.tile_pool(name="xp", bufs=8) as xp,
            tc.tile_pool(name="pp", bufs=2, space="PSUM") as pp,
            tc.tile_pool(name="sp", bufs=2, space="PSUM") as sp,
        ):
            # ---- DMA'd constants ----
            wq_sb = consts.tile([P, CE // P, CH], BF16, tag="wq")
            wk_sb = consts.tile([P, CE // P, CH], BF16, tag="wk")
            wv_sb = consts.tile([P, CE // P, CH], BF16, tag="wv")
            nc.sync.dma_start(wq_sb[:], wq.rearrange("(o p) f -> p o f", p=P))
            nc.sync.dma_start(wk_sb[:], wk.rearrange("(o p) f -> p o f", p=P))
            nc.sync.dma_start(wv_sb[:], wv.rearrange("(o p) f -> p o f", p=P))
            tmask = consts.tile([P, P], BF16, tag="tmask")
            nc.sync.dma_start(tmask[:], tailmask[:])

            # ---- gpsimd-built mask constants ----
            # Atri[ch, p] = 1 if ch < p else 0; Bneg[ch, c] = M0 if c <= ch
            # => (Atri^T @ Bneg)[p, c] = M0 * max(0, p - c): the causal mask.
            ones = consts.tile([P, P], BF16, tag="ones")
            nc.gpsimd.memset(ones[:], 1.0)
            atri = consts.tile([P, P], BF16, tag="atri")
            nc.gpsimd.memset(atri[:], 1.0)
            nc.gpsimd.affine_select(
                out=atri[:],
                in_=atri[:],
                compare_op=mybir.AluOpType.is_ge,
                fill=0.0,
                base=-1,
                pattern=[[1, P]],
                channel_multiplier=-1,
            )
            bneg = consts.tile([P, 2 * P], BF16, tag="bneg")
            nc.gpsimd.memset(bneg[:], M0)
            nc.gpsimd.affine_select(
                out=bneg[:],
                in_=bneg[:],
                compare_op=mybir.AluOpType.is_ge,
                fill=0.0,
                base=0,
                pattern=[[-1, 2 * P]],
                channel_multiplier=1,
            )

            # ---- persistent activations ----
            qT = qkv.tile([CH, T], BF16, tag="qT")
            kTl = qkv.tile([CH, NB * P], BF16, tag="kTl")
            vT = qkv.tile([CH, NB * P], BF16, tag="vT")
            vnat = qkv.tile([P, NB, CH], BF16, tag="vnat")
            stats = qkv.tile([P, NB, 4], F32, tag="stats")
            ssum = qkv.tile([P, NB], F32, tag="ssum")
            rr = qkv.tile([P, NB], F32, tag="rr")
            outsb = qkv.tile([CH, T], F32, tag="outsb")

            w2 = [
                w2p.tile([P, T - 256 * i], BF16, tag=f"w2_{i}", name=f"w2_{i}")
                for i in range(NB)
            ]

            # PE warm-up spam: keeps the HAM clock-gate open while the first
            # input DMAs land (also absorbs the gpsimd-consts wait).
            for t in range(40):
                dscr = sp.tile([CH, 512], F32, tag="po", name=f"warm{t}")
                nc.tensor.matmul(
                    dscr[:, 0:1], ones[0:CH, 0:CH], ones[0:CH, 0:1],
                    start=True, stop=True,
                )
            dscr = sp.tile([CH, 512], F32, tag="po", name="abs_tm")
            nc.tensor.matmul(
                dscr[0:1, 0:1], tmask[0:CH, 0:1], tmask[0:CH, 0:1],
                start=True, stop=True,
            )

            def emit_block(i):
                lhs = kTl[:, P * i : P * (i + 1)]
                qlo = 256 * i
                L = T - qlo
                net = (L + ETILE - 1) // ETILE
                for t3 in range(net):
                    w3 = min(ETILE, L - ETILE * t3)
                    last3 = t3 == net - 1
                    sc = sp.tile([P, ETILE], F32, tag="sc")
                    nsub = (w3 + 511) // 512
                    for u in range(nsub):
                        wu = min(512, w3 - 512 * u)
                        qs = qlo + ETILE * t3 + 512 * u
                        # each 512-col sub-mm fills its own PSUM bank:
                        # start=True per bank (start clears only the
                        # addressed bank's has_written bits)
                        nc.tensor.matmul(
                            sc[:, 512 * u : 512 * u + wu],
                            lhs,
                            qT[:, qs : qs + wu],
                            start=True,
                            stop=(u == nsub - 1 and t3 != 0 and not last3),
                            skip_group_check=True,
                        )
                    if t3 == 0:
                        # causal mask: += M0 * max(0, p - col)
                        nc.tensor.matmul(
                            sc[:, 0:256],
                            atri[:],
                            bneg[:],
                            start=False,
                            stop=not last3,
                        )
                    if last3:
                        # zero-pad tail kill on the final 128 columns
                        nc.tensor.matmul(
                            sc[:, w3 - P : w3],
                            ones[:],
                            tmask[:],
                            start=False,
                            stop=True,
                        )
                    nc.scalar.activation(
                        w2[i][:, ETILE * t3 : ETILE * t3 + w3],
                        sc[:, :w3],
                        mybir.ActivationFunctionType.Exp,
                        scale=SCALE,
                        accum_out=stats[:, i, t3 : t3 + 1],
                    )
                nc.vector.reduce_sum(
                    ssum[:, i : i + 1],
                    stats[:, i, 0:net],
                    axis=mybir.AxisListType.X,
                )
                nc.vector.reciprocal(rr[:, i : i + 1], ssum[:, i : i + 1])
                nc.vector.tensor_scalar_mul(
                    vnat[:, i, :], vnat[:, i, :], rr[:, i : i + 1]
                )

            # ======== merged pipeline: chunks descending, scores inline ========
            def process_chunk(j):
                xtile = xp.tile([P, CE // P, 512], BF16, tag="xtile")
                dma_eng = nc.sync if j % 2 == 0 else nc.scalar
                dma_eng.dma_start(
                    xtile[:],
                    xT[:, 512 * j : 512 * (j + 1)].rearrange(
                        "(o p) f -> p o f", p=P
                    ),
                )
                # absorber: put this chunk's DMA wait on a throwaway MM
                dscr = sp.tile([CH, 512], F32, tag="po", name=f"absx{j}")
                nc.tensor.matmul(
                    dscr[0:1, 0:1],
                    xtile[:, 0, 0:1],
                    xtile[:, 0, 0:1],
                    start=True,
                    stop=True,
                )

                # q projection: full 512 columns
                psq = pp.tile([CH, 512], F32, tag="proj")
                for s in range(CE // P):
                    nc.tensor.matmul(
                        psq[:],
                        wq_sb[:, s, :],
                        xtile[:, s, :],
                        start=(s == 0),
                        stop=(s == CE // P - 1),
                    )
                nc.vector.tensor_copy(qT[:, 512 * j : 512 * (j + 1)], psq[:])

                # k/v: the chunk's two key blocks (offsets 0 and 256) in ONE
                # matmul per subtile via a strided moving operand; k and v
                # share one psum tile (k in [0:256], v in [256:512])
                pskv = pp.tile([CH, 512], F32, tag="proj")
                for s in range(CE // P):
                    kvrhs = xtile[:, s, :].rearrange(
                        "p (b c) -> p b c", c=P
                    )[:, 0::2, :]
                    nc.tensor.matmul(
                        pskv[:, 0:256].rearrange("p (b c) -> p b c", c=P),
                        wk_sb[:, s, :],
                        kvrhs,
                        start=(s == 0),
                        stop=False,
                        skip_group_check=True,
                    )
                for s in range(CE // P):
                    kvrhs = xtile[:, s, :].rearrange(
                        "p (b c) -> p b c", c=P
                    )[:, 0::2, :]
                    nc.tensor.matmul(
                        pskv[:, 256:512].rearrange("p (b c) -> p b c", c=P),
                        wv_sb[:, s, :],
                        kvrhs,
                        start=False,
                        stop=(s == CE // P - 1),
                        skip_group_check=True,
                    )
                nc.vector.tensor_copy(kTl[:, 256 * j : 256 * (j + 1)], pskv[:, 0:256])
                nc.vector.tensor_copy(vT[:, 256 * j : 256 * (j + 1)], pskv[:, 256:512])

                # v -> natural layout via DMA xbar transpose (bf16, no PE)
                for half in (0, 1):
                    i = 2 * j + half
                    nc.sync.dma_start_transpose(
                        vnat[:, i, :], vT[:, P * i : P * (i + 1)]
                    )

                emit_block(2 * j)
                emit_block(2 * j + 1)

            for j in reversed(range(NCHUNK)):
                process_chunk(j)


            # ============ output tail: out^T = sum_i vr_i^T @ w2_i ============
            for c in range(NCHUNK):
                po = sp.tile([CH, 512], F32, tag="po", name=f"po{c}")
                ilast = min(2 * c + 1, NB - 1)
                for i in range(ilast + 1):
                    off = 512 * c - 256 * i
                    if off >= 0:
                        nc.tensor.matmul(
                            po[:],
                            vnat[:, i, :],
                            w2[i][:, off : off + 512],
                            start=(i == 0),
                            stop=(i == ilast),
                        )
                    else:
                        nc.tensor.matmul(
                            po[:, 256:512],
                            vnat[:, i, :],
                            w2[i][:, 0:256],
                            start=False,
                            stop=(i == ilast),
                        )
                nc.vector.tensor_copy(outsb[:, 512 * c : 512 * (c + 1)], po[:])
                nc.sync.dma_start(
                    outT[:, 512 * c : 512 * (c + 1)],
                    outsb[:, 512 * c : 512 * (c + 1)],
                )

    return nc


_PROGRAM = None


def _get_program():
    global _PROGRAM
    if _PROGRAM is None:
        nc = _build_program()
        nc.finalize()
        _PROGRAM = nc
    return _PROGRAM


def kernel(x, Wk, Wq, Wv, trace=False, trace_cores=None):
    global LAST_RESULTS
    x = np.asarray(x)
    Wk = np.asarray(Wk)
    Wq = np.asarray(Wq)
    Wv = np.asarray(Wv)

    import ml_dtypes

    bf = ml_dtypes.bfloat16
    wq_b = Wq.astype(bf)
    wk_b = Wk.astype(bf)
    wv_b = Wv.astype(bf)

    zeros_mask = np.zeros((P, P), bf)
    neg_mask = np.full((P, P), NEG / P, bf)

    in_maps = []
    for c in range(N_CORES):
        b, parity = c // 2, c % 2
        xTb = np.ascontiguousarray(x[b].T).astype(bf)  # [CE, T]
        if parity:
            xTb = np.concatenate([xTb[:, P:], np.zeros((CE, P), bf)], axis=1)
        in_maps.append(
            {
                "xT": np.ascontiguousarray(xTb),
                "wq": wq_b,
                "wk": wk_b,
                "wv": wv_b,
                "tailmask": neg_mask if parity else zeros_mask,
            }
        )

    nc = _get_program()
    res = run_bass_kernel_spmd(
        nc,
        in_maps,
        list(range(N_CORES)),
        trace=trace,
        **({"trace_cores": trace_cores} if trace_cores is not None else {}),
    )
    LAST_RESULTS = res

    out = np.zeros((B, T, CH), np.float32)
    for c in range(N_CORES):
        b, parity = c // 2, c % 2
        oT = np.asarray(res.results[c]["outT"], np.float32)  # [CH, T]
        if parity:
            # core's column t corresponds to q = t + 128
            out[b, P:, :] += oT[:, : T - P].T
        else:
            out[b] += oT.T
    return out



# revision 3
# speedup vs baseline: 1.0367x; 1.0367x over previous
"""Causal self-attention head (softmax over the QUERY axis) on 8 trn2 cores.

Reference math (softmax axis=-2, i.e. per key-column):
    q = x @ Wq; k = x @ Wk; v = x @ Wv            # [B,T,64]
    s[b,q,k] = (q . k) * 64**-0.5, masked to q >= k
    w[:, k]  = softmax over q of s[:, k]           # column softmax
    out[b,q,:] = sum_k w[q,k] v[k,:]

The softmax normalizes over q, so the normalizer folds into per-key scaling:
    out[q] = sum_{k<=q} exp(s[q,k]) * (r[k] * v[k]),  r[k] = 1/sum_{q>=k} exp(s[q,k])

Sharding: 8 cores = 4 batches x 2 "parities". Core (b, p) owns key blocks
2i+p (128 keys each); parity-1 cores get x^T pre-shifted by 128 cols
(zero-pad tail killed by a tailmask matmul); host shifts output back.

v2 kernel structure (per core, pairs j = 7..0, pair = key blocks 2j/2j+1):
- proj: ONE [Wq||Wk] matmul per contraction subtile (M=128): psum rows 0-63
  = q, rows 64-127 = k, for all 512 chunk cols. One DVE cast evacuates both;
  the core's own 2x128 key cols of k are relocated to partitions 0-63 by a
  tiny SBUF->SBUF DMA (scores need lhs/rhs on the same partitions).
- v is projected directly into natural [key, ch] layout via lhsT=x-chunk,
  rhs=Wv (N=64 matmuls) -- no DMA transposes at all.
- scores: K=64 M=128 matmuls into [128,1024] psum groups; causal diag via a
  triangular-count matmul; exp on ACT with accum_out colsums (fp32).
- output: streamed per pair with COL-TILED matmul pairs (tile_position
  (0,0)/(0,64)): even-block partial in psum rows 0-63, odd in 64-127,
  concurrently (2x PE throughput). DVE accumulates into an SBUF [128,T]
  accumulator; the even/odd halves are summed on the HOST (outT is [128,T]).
- odd blocks skip their 256 dead columns (w2 zero prefix via gpsimd memset
  instead of exp of -inf).
"""

import os
import sys
from math import ceil

import numpy as np

for _p in ("/opt/trn_rl_repo",):
    if _p not in sys.path:
        sys.path.insert(0, _p)

import concourse.bass as bass
import concourse.mybir as mybir
from concourse import bacc
from concourse.bass_utils import run_bass_kernel_spmd
from concourse.tile import TileContext

B, T, CE, CH = 4, 4096, 1024, 64
P = 128
NB = 16          # key blocks per core (128 keys each)
NP = 8           # pairs (512-col chunks)
SCALE = CH ** -0.5
NEG = -1e30
M0 = NEG / P     # per-unit magnitude for the triangular-count mask
ETILE = 1024     # scores psum group width (2 banks)

F32 = mybir.dt.float32
BF16 = mybir.dt.bfloat16

N_CORES = 8

LAST_RESULTS = None


def _build_program():
    nc = bacc.Bacc("TRN2", target_bir_lowering=False, debug=False)

    xT = nc.declare_dram_parameter("xT", [CE, T], BF16, isOutput=False)
    wqk = nc.declare_dram_parameter("wqk", [CE, P], BF16, isOutput=False)
    wv = nc.declare_dram_parameter("wv", [CE, CH], BF16, isOutput=False)
    tailmask = nc.declare_dram_parameter("tailmask", [P, P], BF16, isOutput=False)
    outT = nc.declare_dram_parameter("outT", [P, T], F32, isOutput=True)

    with TileContext(nc) as tc:
        with (
            tc.tile_pool(name="consts", bufs=1) as consts,
            tc.tile_pool(name="qkv", bufs=1) as qkv,
            tc.tile_pool(name="w2p", bufs=1) as w2p,
            tc.tile_pool(name="xp", bufs=3) as xp,
            tc.tile_pool(name="pp", bufs=1, space="PSUM") as pp,
            tc.tile_pool(name="sp", bufs=2, space="PSUM") as sp,
            tc.tile_pool(name="op", bufs=2, space="PSUM") as op,
        ):
            # ---- DMA'd constants ----
            wqk_sb = consts.tile([P, CE // P, P], BF16, tag="wqk")
            wv_sb = consts.tile([P, CE // P, CH], BF16, tag="wv")
            nc.sync.dma_start(wqk_sb[:], wqk.rearrange("(o p) f -> p o f", p=P))
            nc.sync.dma_start(wv_sb[:], wv.rearrange("(o p) f -> p o f", p=P))
            tmask = consts.tile([P, P], BF16, tag="tmask")
            nc.sync.dma_start(tmask[:], tailmask[:])

            # ---- gpsimd-built mask constants ----
            # atri[ch, p] = 1 if ch < p; bneg[ch, c] = M0 if c <= ch
            # => (atri^T @ bneg)[p, c] = M0 * max(0, p - c)
            ones = consts.tile([P, P], BF16, tag="ones")
            nc.gpsimd.memset(ones[:], 1.0)
            atri = consts.tile([P, P], BF16, tag="atri")
            nc.gpsimd.memset(atri[:], 1.0)
            nc.gpsimd.affine_select(
                out=atri[:],
                in_=atri[:],
                compare_op=mybir.AluOpType.is_ge,
                fill=0.0,
                base=-1,
                pattern=[[1, P]],
                channel_multiplier=-1,
            )
            bneg = consts.tile([P, P], BF16, tag="bneg")
            nc.gpsimd.memset(bneg[:], M0)
            nc.gpsimd.affine_select(
                out=bneg[:],
                in_=bneg[:],
                compare_op=mybir.AluOpType.is_ge,
                fill=0.0,
                base=0,
                pattern=[[-1, P]],
                channel_multiplier=1,
            )

            # ---- persistent activations ----
            kq_sb = qkv.tile([P, T], BF16, tag="kq_sb")    # rows 0-63 q, 64-127 k-stage
            kTl = qkv.tile([CH, NB * P], BF16, tag="kTl")  # k blocks at parts 0-63
            vnat = qkv.tile([P, NB, CH], F32, tag="vnat")
            vsc = qkv.tile([P, NB, CH], BF16, tag="vsc")
            stats = qkv.tile([P, NB, 4], F32, tag="stats")
            ssum = qkv.tile([P, NB], F32, tag="ssum")
            rr = qkv.tile([P, NB], F32, tag="rr")
            outacc = qkv.tile([P, T], F32, tag="outacc")

            w2 = [
                w2p.tile([P, T - 512 * (i // 2)], BF16, tag=f"w2_{i}", name=f"w2_{i}")
                for i in range(NB)
            ]
            # odd blocks: first 256 cols are a zero prefix (dead causal zone)
            for i in range(1, NB, 2):
                nc.gpsimd.memset(w2[i][:, 0:256], 0.0)

            # PE warm-up spam: keeps the HAM clock-gate open while the first
            # input DMAs land.
            for t in range(40):
                dscr = op.tile([P, 512], F32, tag="po", name=f"warm{t}")
                nc.tensor.matmul(
                    dscr[:, 0:1], ones[:, 0:P], ones[:, 0:1],
                    start=True, stop=True,
                )
            dscr = op.tile([P, 512], F32, tag="po", name="abs_tm")
            nc.tensor.matmul(
                dscr[0:1, 0:1], tmask[:, 0:1], tmask[:, 0:1],
                start=True, stop=True,
            )

            def emit_block(i):
                j = i // 2
                odd = i % 2
                qlo = 512 * j + 256 * odd   # first live q col for this block
                L = T - qlo                  # number of exp cols
                woff = 256 * odd             # col in w2[i] where q=qlo lands
                lhs = kTl[:, P * i : P * (i + 1)]
                ngr = ceil(L / ETILE)
                for g in range(ngr):
                    gw = min(ETILE, L - ETILE * g)
                    sc = sp.tile([P, ETILE], F32, tag="sc")
                    nsub = ceil(gw / 512)
                    for u in range(nsub):
                        wu = min(512, gw - 512 * u)
                        qs = qlo + ETILE * g + 512 * u
                        # bank u gets the diag mask iff (g==0 and u==0);
                        # the tail-kill iff last group and u is last bank
                        has_diag = (g == 0 and u == 0)
                        has_tail = (g == ngr - 1 and u == nsub - 1)
                        nc.tensor.matmul(
                            sc[:, 512 * u : 512 * u + wu],
                            lhs,
                            kq_sb[0:CH, qs : qs + wu],
                            start=True,
                            stop=not (has_diag or has_tail),
                            skip_group_check=True,
                        )
                        if has_diag:
                            nc.tensor.matmul(
                                sc[:, 0:P],
                                atri[:],
                                bneg[:],
                                start=False,
                                stop=not has_tail,
                                skip_group_check=True,
                            )
                        if has_tail:
                            nc.tensor.matmul(
                                sc[:, gw - P : gw],
                                ones[:],
                                tmask[:],
                                start=False,
                                stop=True,
                                skip_group_check=True,
                            )
                    nc.scalar.activation(
                        w2[i][:, woff + ETILE * g : woff + ETILE * g + gw],
                        sc[:, :gw],
                        mybir.ActivationFunctionType.Exp,
                        scale=SCALE,
                        accum_out=stats[:, i, g : g + 1],
                    )
                nc.vector.reduce_sum(
                    ssum[:, i : i + 1],
                    stats[:, i, 0:ngr],
                    axis=mybir.AxisListType.X,
                )
                nc.vector.reciprocal(rr[:, i : i + 1], ssum[:, i : i + 1])
                nc.gpsimd.tensor_scalar_mul(
                    vsc[:, i, :], vnat[:, i, :], rr[:, i : i + 1]
                )

            # ======== pipeline: pairs descending ========
            for j in reversed(range(NP)):
                xtile = xp.tile([P, CE // P, 512], BF16, tag="xtile")
                dma_eng = nc.sync if j % 2 == 0 else nc.scalar
                dma_eng.dma_start(
                    xtile[:],
                    xT[:, 512 * j : 512 * (j + 1)].rearrange(
                        "(o p) f -> p o f", p=P
                    ),
                )
                # absorber: put this chunk's DMA wait on a throwaway MM
                dscr = op.tile([P, 512], F32, tag="po", name=f"absx{j}")
                nc.tensor.matmul(
                    dscr[0:1, 0:1],
                    xtile[:, 0, 0:1],
                    xtile[:, 0, 0:1],
                    start=True,
                    stop=True,
                )

                # q||k projection: psum rows 0-63 = q, 64-127 = k (512 cols)
                kqps = pp.tile([P, 512], F32, tag="kqps")
                for s in range(CE // P):
                    nc.tensor.matmul(
                        kqps[:],
                        wqk_sb[:, s, :],
                        xtile[:, s, :],
                        start=(s == 0),
                        stop=(s == CE // P - 1),
                    )
                nc.vector.tensor_copy(kq_sb[:, 512 * j : 512 * (j + 1)], kqps[:])
                # relocate own k cols (0:128, 256:384 of chunk) to parts 0-63
                nc.gpsimd.dma_start(
                    kTl[:, 256 * j : 256 * (j + 1)].rearrange(
                        "p (b c) -> p b c", c=P
                    ),
                    kq_sb[CH:P, 512 * j : 512 * (j + 1)].rearrange(
                        "p (b c) -> p b c", c=P
                    )[:, 0::2, :],
                )

                # v directly in natural [key, ch] layout: lhsT = x key cols
                vps = pp.tile([P, P], F32, tag="vps")
                for s in range(CE // P):
                    nc.tensor.matmul(
                        vps[:, 0:CH],
                        xtile[:, s, 0:P],
                        wv_sb[:, s, :],
                        start=(s == 0),
                        stop=(s == CE // P - 1),
                        skip_group_check=True,
                    )
                for s in range(CE // P):
                    nc.tensor.matmul(
                        vps[:, CH:P],
                        xtile[:, s, 256 : 256 + P],
                        wv_sb[:, s, :],
                        start=(s == 0),
                        stop=(s == CE // P - 1),
                        skip_group_check=True,
                    )
                nc.vector.tensor_copy(
                    vnat[:, 2 * j : 2 * j + 2, :].rearrange("p b c -> p (b c)"),
                    vps[:],
                )

                emit_block(2 * j)
                emit_block(2 * j + 1)

                # ---- streamed output for pair j (col-tiled even/odd) ----
                for t in range(NP - j):
                    o = 512 * j + 512 * t
                    po = op.tile([P, 512], F32, tag="po", name=f"po{j}_{t}")
                    nc.tensor.matmul(
                        po[0:CH, :],
                        vsc[:, 2 * j, :],
                        w2[2 * j][:, 512 * t : 512 * t + 512],
                        start=True,
                        stop=True,
                        skip_group_check=True,
                    )
                    nc.tensor.matmul(
                        po[CH:P, :],
                        vsc[:, 2 * j + 1, :],
                        w2[2 * j + 1][:, 512 * t : 512 * t + 512],
                        start=True,
                        stop=True,
                        skip_group_check=True,
                    )
                    if t == 0:
                        nc.vector.tensor_copy(outacc[:, o : o + 512], po[:])
                    else:
                        nc.vector.scalar_tensor_tensor(
                            outacc[:, o : o + 512],
                            po[:],
                            1.0,
                            outacc[:, o : o + 512],
                            mybir.AluOpType.bypass,
                            mybir.AluOpType.add,
                        )
                    if j == 0:
                        # region o is final once pair 0 lands; stream it out
                        dma_eng2 = nc.sync if t % 2 == 0 else nc.scalar
                        dma_eng2.dma_start(
                            outT[:, o : o + 512], outacc[:, o : o + 512]
                        )

    return nc


_PROGRAM = None


def _get_program():
    global _PROGRAM
    if _PROGRAM is None:
        nc = _build_program()
        nc.finalize()
        _PROGRAM = nc
    return _PROGRAM


def kernel(x, Wk, Wq, Wv, trace=False, trace_cores=None):
    global LAST_RESULTS
    x = np.asarray(x)
    Wk = np.asarray(Wk)
    Wq = np.asarray(Wq)
    Wv = np.asarray(Wv)

    import ml_dtypes

    bf = ml_dtypes.bfloat16
    wqk_b = np.concatenate([Wq, Wk], axis=1).astype(bf)  # [CE, 128]
    wv_b = Wv.astype(bf)

    zeros_mask = np.zeros((P, P), bf)
    neg_mask = np.full((P, P), NEG / P, bf)

    in_maps = []
    for c in range(N_CORES):
        b, parity = c // 2, c % 2
        xTb = np.ascontiguousarray(x[b].T).astype(bf)  # [CE, T]
        if parity:
            xTb = np.concatenate([xTb[:, P:], np.zeros((CE, P), bf)], axis=1)
        in_maps.append(
            {
                "xT": np.ascontiguousarray(xTb),
                "wqk": wqk_b,
                "wv": wv_b,
                "tailmask": neg_mask if parity else zeros_mask,
            }
        )

    nc = _get_program()
    res = run_bass_kernel_spmd(
        nc,
        in_maps,
        list(range(N_CORES)),
        trace=trace,
        **({"trace_cores": trace_cores} if trace_cores is not None else {}),
    )
    LAST_RESULTS = res

    out = np.zeros((B, T, CH), np.float32)
    for c in range(N_CORES):
        b, parity = c // 2, c % 2
        oTf = np.asarray(res.results[c]["outT"], np.float32)  # [128, T]
        oT = oTf[0:CH] + oTf[CH:P]  # fold even/odd block halves
        if parity:
            out[b, P:, :] += oT[:, : T - P].T
        else:
            out[b] += oT.T
    return out


# revision 7
# speedup vs baseline: 1.1566x; 1.1157x over previous
"""Causal self-attention head (softmax over the QUERY axis) on 8 trn2 cores.

Reference math (softmax axis=-2, i.e. per key-column):
    q = x @ Wq; k = x @ Wk; v = x @ Wv            # [B,T,64]
    s[b,q,k] = (q . k) * 64**-0.5, masked to q >= k
    w[:, k]  = softmax over q of s[:, k]           # column softmax
    out[b,q,:] = sum_k w[q,k] v[k,:]

The softmax normalizes over q, so the normalizer folds into per-key scaling:
    out[q] = sum_{k<=q} exp(s[q,k]) * (r[k] * v[k]),  r[k] = 1/sum_{q>=k} exp(s[q,k])

Sharding: 8 cores = 4 batches x 2 "parities". Core (b, p) owns key blocks
2i+p (128 keys each); parity-1 cores get x^T pre-shifted by 128 cols
(zero-pad tail killed by a tailmask matmul); host shifts output back.

v2 kernel structure (per core, pairs j = 7..0, pair = key blocks 2j/2j+1):
- proj: ONE [Wq||Wk] matmul per contraction subtile (M=128): psum rows 0-63
  = q, rows 64-127 = k, for all 512 chunk cols. One DVE cast evacuates both;
  the core's own 2x128 key cols of k are relocated to partitions 0-63 by a
  tiny SBUF->SBUF DMA (scores need lhs/rhs on the same partitions).
- v is projected directly into natural [key, ch] layout via lhsT=x-chunk,
  rhs=Wv (N=64 matmuls) -- no DMA transposes at all.
- scores: K=64 M=128 matmuls into [128,1024] psum groups; causal diag via a
  triangular-count matmul; exp on ACT with accum_out colsums (fp32).
- output: streamed per pair with COL-TILED matmul pairs (tile_position
  (0,0)/(0,64)): even-block partial in psum rows 0-63, odd in 64-127,
  concurrently (2x PE throughput). DVE accumulates into an SBUF [128,T]
  accumulator; the even/odd halves are summed on the HOST (outT is [128,T]).
- odd blocks skip their 256 dead columns (w2 zero prefix via gpsimd memset
  instead of exp of -inf).
"""

import os
import sys
from math import ceil

import numpy as np

for _p in ("/opt/trn_rl_repo",):
    if _p not in sys.path:
        sys.path.insert(0, _p)

import concourse.bass as bass
import concourse.mybir as mybir
from concourse import bacc
from concourse.bass_utils import run_bass_kernel_spmd
from concourse.tile import TileContext

B, T, CE, CH = 4, 4096, 1024, 64
P = 128
NB = 16          # key blocks per core (128 keys each)
NP = 8           # pairs (512-col chunks)
SCALE = CH ** -0.5
NEG = -1e30
M0 = NEG / P     # per-unit magnitude for the triangular-count mask
ETILE = 1024     # scores psum group width (2 banks)

F32 = mybir.dt.float32
BF16 = mybir.dt.bfloat16

N_CORES = 8

LAST_RESULTS = None


def _build_program():
    nc = bacc.Bacc("TRN2", target_bir_lowering=False, debug=False)

    xT = nc.declare_dram_parameter("xT", [CE, T], BF16, isOutput=False)
    wqk = nc.declare_dram_parameter("wqk", [CE, P], BF16, isOutput=False)
    wv = nc.declare_dram_parameter("wv", [CE, CH], BF16, isOutput=False)
    tailmask = nc.declare_dram_parameter("tailmask", [P, P], BF16, isOutput=False)
    outT = nc.declare_dram_parameter("outT", [P, T], F32, isOutput=True)

    with TileContext(nc) as tc:
        with (
            tc.tile_pool(name="consts", bufs=1) as consts,
            tc.tile_pool(name="qkv", bufs=1) as qkv,
            tc.tile_pool(name="w2p", bufs=1) as w2p,
            tc.tile_pool(name="xp", bufs=3) as xp,
            tc.tile_pool(name="pp", bufs=1, space="PSUM") as pp,
            tc.tile_pool(name="sp", bufs=2, space="PSUM") as sp,
            tc.tile_pool(name="op", bufs=2, space="PSUM") as op,
        ):
            # ---- DMA'd constants ----
            wqk_sb = consts.tile([P, CE // P, P], BF16, tag="wqk")
            wv_sb = consts.tile([P, CE // P, CH], BF16, tag="wv")
            nc.sync.dma_start(wqk_sb[:], wqk.rearrange("(o p) f -> p o f", p=P))
            nc.sync.dma_start(wv_sb[:], wv.rearrange("(o p) f -> p o f", p=P))
            tmask = consts.tile([P, P], BF16, tag="tmask")
            nc.sync.dma_start(tmask[:], tailmask[:])

            # ---- gpsimd-built mask constants ----
            # atri[ch, p] = 1 if ch < p; bneg[ch, c] = M0 if c <= ch
            # => (atri^T @ bneg)[p, c] = M0 * max(0, p - c)
            ones = consts.tile([P, P], BF16, tag="ones")
            nc.gpsimd.memset(ones[:], 1.0)
            atri = consts.tile([P, P], BF16, tag="atri")
            nc.gpsimd.memset(atri[:], 1.0)
            nc.gpsimd.affine_select(
                out=atri[:],
                in_=atri[:],
                compare_op=mybir.AluOpType.is_ge,
                fill=0.0,
                base=-1,
                pattern=[[1, P]],
                channel_multiplier=-1,
            )
            bneg = consts.tile([P, P], BF16, tag="bneg")
            nc.gpsimd.memset(bneg[:], M0)
            nc.gpsimd.affine_select(
                out=bneg[:],
                in_=bneg[:],
                compare_op=mybir.AluOpType.is_ge,
                fill=0.0,
                base=0,
                pattern=[[-1, P]],
                channel_multiplier=1,
            )

            # ---- persistent activations ----
            kq_sb = qkv.tile([P, T], BF16, tag="kq_sb")    # rows 0-63 q, 64-127 k-stage
            kTl = qkv.tile([CH, NB * P], BF16, tag="kTl")  # k blocks at parts 0-63
            vnat = qkv.tile([P, NB, CH], F32, tag="vnat")
            vsc = qkv.tile([P, NB, CH], BF16, tag="vsc")
            stats = qkv.tile([P, NB, 4], F32, tag="stats")
            ssum = qkv.tile([P, NB], F32, tag="ssum")
            rr = qkv.tile([P, NB], F32, tag="rr")
            outacc = qkv.tile([P, T], F32, tag="outacc")

            w2 = [
                w2p.tile([P, T - 512 * (i // 2)], BF16, tag=f"w2_{i}", name=f"w2_{i}")
                for i in range(NB)
            ]
            # odd blocks: first 256 cols are a zero prefix (dead causal zone)
            for i in range(1, NB, 2):
                nc.gpsimd.memset(w2[i][:, 0:256], 0.0)

            # PE warm-up spam: keeps the HAM clock-gate open while the first
            # input DMAs land.
            for t in range(48):
                dscr = op.tile([P, 512], F32, tag="po", name=f"warm{t}")
                nc.tensor.matmul(
                    dscr[:, 0:P], ones[:, 0:P], ones[:, 0:P],
                    start=True, stop=True,
                )
            dscr = op.tile([P, 512], F32, tag="po", name="abs_tm")
            nc.tensor.matmul(
                dscr[0:1, 0:1], tmask[:, 0:1], tmask[:, 0:1],
                start=True, stop=True,
            )

            def emit_block(i):
                j = i // 2
                odd = i % 2
                qlo = 512 * j + 256 * odd   # first live q col for this block
                L = T - qlo                  # number of exp cols
                woff = 256 * odd             # col in w2[i] where q=qlo lands
                lhs = kTl[:, P * i : P * (i + 1)]
                ngr = ceil(L / ETILE)
                for g in range(ngr):
                    gw = min(ETILE, L - ETILE * g)
                    sc = sp.tile([P, ETILE], F32, tag="sc")
                    nsub = ceil(gw / 512)
                    for u in range(nsub):
                        wu = min(512, gw - 512 * u)
                        qs = qlo + ETILE * g + 512 * u
                        # bank u gets the diag mask iff (g==0 and u==0);
                        # the tail-kill iff last group and u is last bank
                        has_diag = (g == 0 and u == 0)
                        has_tail = (g == ngr - 1 and u == nsub - 1)
                        nc.tensor.matmul(
                            sc[:, 512 * u : 512 * u + wu],
                            lhs,
                            kq_sb[0:CH, qs : qs + wu],
                            start=True,
                            stop=not (has_diag or has_tail),
                            skip_group_check=True,
                        )
                        if has_diag:
                            nc.tensor.matmul(
                                sc[:, 0:P],
                                atri[:],
                                bneg[:],
                                start=False,
                                stop=not has_tail,
                                skip_group_check=True,
                            )
                        if has_tail:
                            nc.tensor.matmul(
                                sc[:, gw - P : gw],
                                ones[:],
                                tmask[:],
                                start=False,
                                stop=True,
                                skip_group_check=True,
                            )
                    nc.scalar.activation(
                        w2[i][:, woff + ETILE * g : woff + ETILE * g + gw],
                        sc[:, :gw],
                        mybir.ActivationFunctionType.Exp,
                        scale=SCALE,
                        accum_out=stats[:, i, g : g + 1],
                    )
                nc.vector.reduce_sum(
                    ssum[:, i : i + 1],
                    stats[:, i, 0:ngr],
                    axis=mybir.AxisListType.X,
                )
                nc.vector.reciprocal(rr[:, i : i + 1], ssum[:, i : i + 1])
                nc.vector.tensor_scalar_mul(
                    vsc[:, i, :], vnat[:, i, :], rr[:, i : i + 1]
                )

            # ---- streamed output for pair j (col-tiled even/odd) ----
            # Emitted one iteration AFTER pair j's exp chain so the PE queue
            # (strict FIFO) never stalls on the ACT->rr->vsc dependency: by
            # the time the PE reaches these matmuls, vsc[j] is long done.
            def emit_output(j):
                for t in range(NP - j):
                    o = 512 * j + 512 * t
                    po = op.tile([P, 512], F32, tag="po", name=f"po{j}_{t}")
                    nc.tensor.matmul(
                        po[0:CH, :],
                        vsc[:, 2 * j, :],
                        w2[2 * j][:, 512 * t : 512 * t + 512],
                        start=True,
                        stop=True,
                        skip_group_check=True,
                    )
                    nc.tensor.matmul(
                        po[CH:P, :],
                        vsc[:, 2 * j + 1, :],
                        w2[2 * j + 1][:, 512 * t : 512 * t + 512],
                        start=True,
                        stop=True,
                        skip_group_check=True,
                    )
                    if t == 0:
                        nc.vector.tensor_copy(outacc[:, o : o + 512], po[:])
                    else:
                        nc.vector.scalar_tensor_tensor(
                            outacc[:, o : o + 512],
                            po[:],
                            1.0,
                            outacc[:, o : o + 512],
                            mybir.AluOpType.bypass,
                            mybir.AluOpType.add,
                        )
                    if j == 0:
                        # region o is final once pair 0 lands; stream it out
                        nc.sync.dma_start(
                            outT[:, o : o + 512], outacc[:, o : o + 512]
                        )

            # ======== pipeline: pairs descending ========
            for j in reversed(range(NP)):
                xtile = xp.tile([P, CE // P, 512], BF16, tag="xtile")
                dma_eng = nc.sync if j % 2 == 0 else nc.gpsimd
                dma_eng.dma_start(
                    xtile[:],
                    xT[:, 512 * j : 512 * (j + 1)].rearrange(
                        "(o p) f -> p o f", p=P
                    ),
                )
                # absorber: put this chunk's DMA wait on a throwaway MM
                dscr = op.tile([P, 512], F32, tag="po", name=f"absx{j}")
                nc.tensor.matmul(
                    dscr[0:1, 0:1],
                    xtile[:, 0, 0:1],
                    xtile[:, 0, 0:1],
                    start=True,
                    stop=True,
                )

                # q||k projection: psum rows 0-63 = q, 64-127 = k (512 cols)
                kqps = pp.tile([P, 512], F32, tag="kqps")
                for s in range(CE // P):
                    nc.tensor.matmul(
                        kqps[:],
                        wqk_sb[:, s, :],
                        xtile[:, s, :],
                        start=(s == 0),
                        stop=(s == CE // P - 1),
                    )
                nc.vector.tensor_copy(kq_sb[:, 512 * j : 512 * (j + 1)], kqps[:])
                # relocate own k cols (0:128, 256:384 of chunk) to parts 0-63
                nc.gpsimd.dma_start(
                    kTl[:, 256 * j : 256 * (j + 1)].rearrange(
                        "p (b c) -> p b c", c=P
                    ),
                    kq_sb[CH:P, 512 * j : 512 * (j + 1)].rearrange(
                        "p (b c) -> p b c", c=P
                    )[:, 0::2, :],
                )

                # v directly in natural [key, ch] layout: lhsT = x key cols
                vps = pp.tile([P, P], F32, tag="vps")
                for s in range(CE // P):
                    nc.tensor.matmul(
                        vps[:, 0:CH],
                        xtile[:, s, 0:P],
                        wv_sb[:, s, :],
                        start=(s == 0),
                        stop=(s == CE // P - 1),
                        skip_group_check=True,
                    )
                for s in range(CE // P):
                    nc.tensor.matmul(
                        vps[:, CH:P],
                        xtile[:, s, 256 : 256 + P],
                        wv_sb[:, s, :],
                        start=(s == 0),
                        stop=(s == CE // P - 1),
                        skip_group_check=True,
                    )
                nc.vector.tensor_copy(
                    vnat[:, 2 * j : 2 * j + 2, :].rearrange("p b c -> p (b c)"),
                    vps[:],
                )

                emit_block(2 * j)
                emit_block(2 * j + 1)

                if j < NP - 1:
                    emit_output(j + 1)
            emit_output(0)

    return nc


_PROGRAM = None


def _get_program():
    global _PROGRAM
    if _PROGRAM is None:
        nc = _build_program()
        nc.finalize()
        _PROGRAM = nc
    return _PROGRAM


def kernel(x, Wk, Wq, Wv, trace=False, trace_cores=None):
    global LAST_RESULTS
    x = np.asarray(x)
    Wk = np.asarray(Wk)
    Wq = np.asarray(Wq)
    Wv = np.asarray(Wv)

    import ml_dtypes

    bf = ml_dtypes.bfloat16
    wqk_b = np.concatenate([Wq, Wk], axis=1).astype(bf)  # [CE, 128]
    wv_b = Wv.astype(bf)

    zeros_mask = np.zeros((P, P), bf)
    neg_mask = np.full((P, P), NEG / P, bf)

    in_maps = []
    for c in range(N_CORES):
        b, parity = c // 2, c % 2
        xTb = np.ascontiguousarray(x[b].T).astype(bf)  # [CE, T]
        if parity:
            xTb = np.concatenate([xTb[:, P:], np.zeros((CE, P), bf)], axis=1)
        in_maps.append(
            {
                "xT": np.ascontiguousarray(xTb),
                "wqk": wqk_b,
                "wv": wv_b,
                "tailmask": neg_mask if parity else zeros_mask,
            }
        )

    nc = _get_program()
    res = run_bass_kernel_spmd(
        nc,
        in_maps,
        list(range(N_CORES)),
        trace=trace,
        **({"trace_cores": trace_cores} if trace_cores is not None else {}),
    )
    LAST_RESULTS = res

    out = np.zeros((B, T, CH), np.float32)
    for c in range(N_CORES):
        b, parity = c // 2, c % 2
        oTf = np.asarray(res.results[c]["outT"], np.float32)  # [128, T]
        oT = oTf[0:CH] + oTf[CH:P]  # fold even/odd block halves
        if parity:
            out[b, P:, :] += oT[:, : T - P].T
        else:
            out[b] += oT.T
    return out
